# revision 3
# baseline (speedup 1.0000x reference)
"""LSTMCell (B=65536, H=512) Bass/Tile kernel for 8 trn2 NeuronCores.

Data-parallel over batch: each core processes 8192 rows (64 tiles of 128).
Inputs are staged to DRAM as bf16 (halves HBM-in traffic); outputs written
bf16 and upcast to fp32 on host (halves HBM-out traffic). Per 128-row tile:
  z = x + stm                      (DVE, bf16)
  zT via XBAR DMA transpose        (DMA ucode, no PE/PSUM involvement)
  gates = zT.T @ W (bf16)          (TensorE, 4 matmuls of 2048 cols, PSUM x2)
  pre = gates + bias               (DVE, one [128,2048] op, drains PSUM)
  sigmoid(f,i,o) / tanh(g) -> bf16 (ACT, two ops)
  prod = si*tg                     (GpSimd)
  c = sf + prod                    (DVE)
  tanh(c)                          (ACT)
  h = tc*so                        (GpSimd)
Engine budget/tile ~ PE 3.4us > ACT 3.0 > DVE 2.9 > GpSimd 2.2 > Sync 2.5.
"""

import os
import sys

if "/opt/trn_rl_repo" not in sys.path:
    sys.path.insert(0, "/opt/trn_rl_repo")

import numpy as np

import concourse.bacc as bacc
import concourse.mybir as mybir
import concourse.tile as tile

N_CORES = 8
B, H = 65536, 512
B_CORE = B // N_CORES  # 8192
F32 = mybir.dt.float32
BF16 = mybir.dt.bfloat16
AF = mybir.ActivationFunctionType

NEFF_DUMP = "/tmp/lstm_kernel.neff"

# gate order in the packed weight/bias/psum layout: sigmoid gates first so one
# ACT op covers [0:1536], tanh gate last at [1536:2048]
#   slot 0: f (sigmoid), 1: i (sigmoid), 2: o (sigmoid), 3: g (tanh)


def build_module(b_core=B_CORE, n_cores=N_CORES):
    nc = bacc.Bacc(
        "TRN2",
        target_bir_lowering=False,
        debug=False,
        num_devices=n_cores,
    )
    xs = nc.dram_tensor("xs", [2, b_core, H], BF16, kind="ExternalInput").ap()
    wt = nc.dram_tensor("wt", [128, 4, 2048], BF16, kind="ExternalInput").ap()
    bias = nc.dram_tensor("bias", [128, 2048], F32, kind="ExternalInput").ap()
    out = nc.dram_tensor("out", [b_core, 2, H], BF16, kind="ExternalOutput").ap()

    n_tiles = b_core // 128

    with tile.TileContext(nc) as tc:
        with (
            tc.tile_pool(name="const", bufs=1) as cpool,
            tc.tile_pool(name="work", bufs=4) as pool,
            tc.tile_pool(name="pgates", bufs=2, space="PSUM") as pg,
        ):
            wt_sb = cpool.tile([128, 4, 2048], BF16)
            nc.sync.dma_start(out=wt_sb[:], in_=wt[:])
            bias_sb = cpool.tile([128, 2048], F32)
            nc.sync.dma_start(out=bias_sb[:], in_=bias[:])

            for t in range(n_tiles):
                rows = slice(t * 128, (t + 1) * 128)
                x_t = pool.tile([128, H], BF16, tag="x")
                nc.sync.dma_start(out=x_t[:], in_=xs[0, rows, :])
                s_t = pool.tile([128, H], BF16, tag="s")
                nc.sync.dma_start(out=s_t[:], in_=xs[1, rows, :])
                z_t = pool.tile([128, H], BF16, tag="z")
                nc.vector.tensor_add(z_t[:], x_t[:], s_t[:])

                # zt[p, c, b] = z[b, c*128 + p] via XBAR DMA transpose
                zt_t = pool.tile([128, 4, 128], BF16, tag="zt")
                nc.sync.dma_start_transpose(zt_t[:], z_t[:])

                # gates[b, gs*512+j] = sum_h z[b,h] * W_gs[j,h], k-chunked.
                # each matmul's psum output must stay within one 2KB bank.
                g_ps = pg.tile([128, 2048], F32, tag="gates")
                for k in range(4):
                    for gs in range(4):
                        nc.tensor.matmul(
                            g_ps[:, gs * 512 : (gs + 1) * 512],
                            zt_t[:, k, :],
                            wt_sb[:, k, gs * 512 : (gs + 1) * 512],
                            start=(k == 0),
                            stop=(k == 3),
                        )

                pre = pool.tile([128, 2048], F32, tag="pre")
                nc.vector.tensor_add(pre[:], g_ps[:], bias_sb[:])
                acts = pool.tile([128, 2048], BF16, tag="acts")
                nc.scalar.activation(acts[:, 0:1536], pre[:, 0:1536], AF.Sigmoid)
                nc.scalar.activation(acts[:, 1536:2048], pre[:, 1536:2048], AF.Tanh)

                prod = pool.tile([128, H], BF16, tag="prod")
                nc.gpsimd.tensor_mul(prod[:], acts[:, 512:1024], acts[:, 1536:2048])
                ch = pool.tile([128, 2, H], BF16, tag="ch")
                nc.vector.tensor_add(ch[:, 0, :], acts[:, 0:512], prod[:])
                tc_t = pool.tile([128, H], BF16, tag="tc")
                nc.scalar.activation(tc_t[:], ch[:, 0, :], AF.Tanh)
                nc.gpsimd.tensor_mul(ch[:, 1, :], tc_t[:], acts[:, 1024:1536])

                nc.sync.dma_start(out=out[rows, :, :], in_=ch[:])

    nc.compile()
    return nc


def pack_inputs(inputs, short_term_memory, Wf, bf, Wi, bi, Wg, bg, Wo, bo):
    import ml_dtypes

    bf16 = ml_dtypes.bfloat16
    x = np.asarray(inputs, np.float32).astype(bf16)
    s = np.asarray(short_term_memory, np.float32).astype(bf16)
    xs = np.stack([x, s], axis=0)  # [2, B, H]
    Ws = [Wf, Wi, Wo, Wg]
    bs = [bf, bi, bo, bg]
    wt = np.empty((128, 4, 2048), bf16)
    for gs, W in enumerate(Ws):
        Wt = np.ascontiguousarray(np.asarray(W, np.float32).T)  # [h, j] = W[j, h]
        # wt[p, kc, gs*512 + j] = W[j, kc*128 + p]
        wt[:, :, gs * 512 : (gs + 1) * 512] = (
            Wt.reshape(4, 128, 512).transpose(1, 0, 2).astype(bf16)
        )
    bias = np.empty((128, 2048), np.float32)
    for gs, b in enumerate(bs):
        bias[:, gs * 512 : (gs + 1) * 512] = np.asarray(b, np.float32)[None, :]
    return {"xs": xs, "wt": wt, "bias": bias}


class Runner:
    """Compiles the module once and keeps a reusable jitted executor."""

    def __init__(self, nc=None, n_cores=N_CORES):
        import jax
        from concourse import bass2jax as b2j

        self.jax = jax
        self.n_cores = n_cores
        self.nc = nc or build_module(n_cores=n_cores)
        b2j.install_neuronx_cc_hook()

        # dump the final (renamed) NEFF so neuron-profile can pair it with NTFFs
        if not getattr(b2j, "_neff_dump_patched", False):
            orig = b2j.rename_neff_tensors_and_patch_header

            def _patched(neff_path, mapping):
                data = orig(neff_path, mapping)
                with open(NEFF_DUMP, "wb") as f:
                    f.write(data)
                return data

            b2j.rename_neff_tensors_and_patch_header = _patched
            b2j._neff_dump_patched = True

        from jax.experimental.shard_map import shard_map
        from jax.sharding import Mesh, NamedSharding, PartitionSpec

        part_name = (
            self.nc.partition_id_tensor.name if self.nc.partition_id_tensor else None
        )
        in_names, out_names, out_avals = [], [], []
        self.out_shapes = {}
        for alloc in self.nc.m.functions[0].allocations:
            if not isinstance(alloc, mybir.MemoryLocationSet):
                continue
            name = alloc.memorylocations[0].name
            if alloc.kind == "ExternalInput":
                if name != part_name:
                    in_names.append(name)
            elif alloc.kind == "ExternalOutput":
                out_names.append(name)
                shape = tuple(alloc.tensor_shape)
                dt = mybir.dt.np(alloc.dtype)
                out_avals.append(jax.core.ShapedArray(shape, dt))
                self.out_shapes[name] = (shape, dt)
        self.in_names, self.out_names = in_names, out_names
        nc_ref = self.nc

        bind_names = list(in_names) + list(out_names)
        if part_name is not None:
            bind_names.append(part_name)

        def _body(*args):
            operands = list(args)
            if part_name is not None:
                operands.append(b2j.partition_id_tensor())
            outs = b2j._bass_exec_p.bind(
                *operands,
                out_avals=tuple(out_avals),
                in_names=tuple(bind_names),
                out_names=tuple(out_names),
                lowering_input_output_aliases=(),
                sim_require_finite=False,
                sim_require_nnan=False,
                nc=nc_ref,
            )
            return tuple(outs)

        devices = jax.devices()[: self.n_cores]
        mesh = Mesh(np.asarray(devices), ("core",))
        spec = PartitionSpec("core")
        n_args = len(in_names) + len(out_names)
        self.sharding = NamedSharding(mesh, spec)
        self.fn = jax.jit(
            shard_map(
                _body,
                mesh=mesh,
                in_specs=(spec,) * n_args,
                out_specs=(spec,) * len(out_names),
                check_rep=False,
            ),
            keep_unused=True,
        )
        self._dev_args = None

    def stage(self, packed):
        """Transfer inputs (sharded/replicated as needed) to devices once."""
        jax = self.jax
        nc_n = self.n_cores
        args = []
        for name in self.in_names:
            a = packed[name]
            if name == "xs":
                # [2, B, H] -> per-core [2, B_CORE, H]: global layout
                # [2*n_cores, B_CORE, H] with core c at rows [2c, 2c+2)
                g = a.reshape(2, nc_n, B_CORE, a.shape[-1])
                glob = np.ascontiguousarray(
                    g.transpose(1, 0, 2, 3).reshape(2 * nc_n, B_CORE, a.shape[-1])
                )
            else:
                glob = np.concatenate([a] * nc_n, axis=0)  # replicate
            args.append(glob)
        for name in self.out_names:
            shape, dt = self.out_shapes[name]
            args.append(np.zeros((shape[0] * nc_n,) + shape[1:], dt))
        self._dev_args = [jax.device_put(a, self.sharding) for a in args]

    def execute(self):
        outs = self.fn(*self._dev_args)
        self.jax.block_until_ready(outs)
        return outs

    def run(self, packed):
        self.stage(packed)
        outs = self.execute()
        res = {}
        for name, arr in zip(self.out_names, outs):
            a = np.asarray(arr)  # [n_cores*d0, ...]
            shape, _ = self.out_shapes[name]
            res[name] = a.reshape((self.n_cores, shape[0]) + tuple(shape[1:]))
        return res


_RUNNER = None


def _get_runner():
    global _RUNNER
    if _RUNNER is None:
        _RUNNER = Runner()
    return _RUNNER


def kernel(**inputs):
    r = _get_runner()
    packed = pack_inputs(**inputs)
    res = r.run(packed)
    per_core = res["out"]  # [8, 8192, 2, 512] bf16
    return np.ascontiguousarray(
        per_core.transpose(2, 0, 1, 3).reshape(2, B, H).astype(np.float32)
    )


if __name__ == "__main__":
    nc = build_module()
    print("module built + compiled OK")


# revision 7
# speedup vs baseline: 1.0670x; 1.0670x over previous
"""LSTMCell (B=65536, H=512) Bass/Tile kernel for 8 trn2 NeuronCores.

Data-parallel over batch: each core processes 8192 rows (64 tiles of 128).
Inputs are staged to DRAM as bf16 (halves HBM-in traffic); outputs written
bf16 and upcast to fp32 on host (halves HBM-out traffic). Per 128-row tile:
  z = x + stm                      (DVE, bf16)
  zT via XBAR DMA transpose        (DMA ucode, no PE/PSUM involvement)
  gates = zT.T @ W (bf16)          (TensorE, 4 matmuls of 2048 cols, PSUM x2)
  pre = gates + bias               (DVE, one [128,2048] op, drains PSUM)
  sigmoid(f,i,o) / tanh(g) -> bf16 (ACT, two ops)
  prod = si*tg                     (GpSimd)
  c = sf + prod                    (DVE)
  tanh(c)                          (ACT)
  h = tc*so                        (GpSimd)
Engine budget/tile ~ PE 3.4us > ACT 3.0 > DVE 2.9 > GpSimd 2.2 > Sync 2.5.
"""

import os
import sys

if "/opt/trn_rl_repo" not in sys.path:
    sys.path.insert(0, "/opt/trn_rl_repo")

import numpy as np

import concourse.bacc as bacc
import concourse.mybir as mybir
import concourse.tile as tile

N_CORES = 8
B, H = 65536, 512
B_CORE = B // N_CORES  # 8192
F32 = mybir.dt.float32
BF16 = mybir.dt.bfloat16
AF = mybir.ActivationFunctionType

NEFF_DUMP = "/tmp/lstm_kernel.neff"

# gate order in the packed weight/bias/psum layout: sigmoid gates first so one
# ACT op covers [0:1536], tanh gate last at [1536:2048]
#   slot 0: f (sigmoid), 1: i (sigmoid), 2: o (sigmoid), 3: g (tanh)


def build_module(b_core=B_CORE, n_cores=N_CORES):
    nc = bacc.Bacc(
        "TRN2",
        target_bir_lowering=False,
        debug=False,
        num_devices=n_cores,
    )
    xs = nc.dram_tensor("xs", [b_core, 2, H], BF16, kind="ExternalInput").ap()
    wt = nc.dram_tensor("wt", [128, 4, 2048], BF16, kind="ExternalInput").ap()
    bias = nc.dram_tensor("bias", [128, 2048], F32, kind="ExternalInput").ap()
    out = nc.dram_tensor("out", [b_core, 2, H], BF16, kind="ExternalOutput").ap()

    T = b_core // 128

    # Software pipeline with stage offsets so every in-order engine queue only
    # sees instructions whose data is (nearly) ready — no head-of-line stalls:
    #   iter t:  Sync: in(t), xbar(t-2), out(t-7)
    #            PE:   matmuls(t-4)
    #            DVE:  evac(t-4), c-add(t-5), z-add(t)
    #            ACT:  sig(t-4), tanhg(t-4), tanhc(t-5)
    #            Gp:   prod(t-4), h-mul(t-5)
    with tile.TileContext(nc) as tc:
        with (
            tc.tile_pool(name="const", bufs=1) as cpool,
            tc.tile_pool(name="pin", bufs=4) as pin,
            tc.tile_pool(name="pz", bufs=5) as pz,
            tc.tile_pool(name="pmid", bufs=4) as pmid,
            tc.tile_pool(name="pout", bufs=5) as pout,
            tc.tile_pool(name="pgates", bufs=2, space="PSUM") as pg,
        ):
            wt_sb = cpool.tile([128, 4, 2048], BF16)
            nc.sync.dma_start(out=wt_sb[:], in_=wt[:])
            bias_sb = cpool.tile([128, 2048], F32)
            nc.sync.dma_start(out=bias_sb[:], in_=bias[:])

            xs_t = [None] * T
            z_t = [None] * T
            zt_t = [None] * T
            acts_t = [None] * T
            prod_t = [None] * T
            ch_t = [None] * T

            for t in range(T + 7):
                # S0: input prefetch (x|s row-interleaved, one DMA)
                if t < T:
                    rows = slice(t * 128, (t + 1) * 128)
                    xs_t[t] = pin.tile([128, 2, H], BF16, tag="xs", name=f"xs{t}")
                    nc.sync.dma_start(out=xs_t[t][:], in_=xs[rows, :, :])

                # S1: matmuls for tile t-4 (psum bank per 512-col gate chunk)
                if 4 <= t < T + 4:
                    m = t - 4
                    g_ps = pg.tile([128, 2048], F32, tag="gates")
                    for k in range(4):
                        for gs in range(4):
                            nc.tensor.matmul(
                                g_ps[:, gs * 512 : (gs + 1) * 512],
                                zt_t[m][:, k, :],
                                wt_sb[:, k, gs * 512 : (gs + 1) * 512],
                                start=(k == 0),
                                stop=(k == 3),
                            )
                    # S2: drain psum (bias add), activations, input-gate product
                    pre = pmid.tile([128, 2048], BF16, tag="pre")
                    nc.vector.tensor_add(pre[:], g_ps[:], bias_sb[:])
                    acts_t[m] = pmid.tile([128, 2048], BF16, tag="acts", name=f"acts{m}")
                    nc.scalar.activation(
                        acts_t[m][:, 0:1536], pre[:, 0:1536], AF.Sigmoid
                    )
                    nc.scalar.activation(
                        acts_t[m][:, 1536:2048], pre[:, 1536:2048], AF.Tanh
                    )
                    prod_t[m] = pmid.tile([128, H], BF16, tag="prod", name=f"prod{m}")
                    nc.gpsimd.tensor_mul(
                        prod_t[m][:], acts_t[m][:, 512:1024], acts_t[m][:, 1536:2048]
                    )

                # S3: tail for tile t-5 (prod long ready -> no DVE queue stall)
                if 5 <= t < T + 5:
                    e = t - 5
                    a = acts_t[e]
                    ch_t[e] = pout.tile([128, 2, H], BF16, tag="ch", name=f"ch{e}")
                    nc.vector.tensor_add(ch_t[e][:, 0, :], a[:, 0:512], prod_t[e][:])
                    tc_t = pmid.tile([128, H], BF16, tag="tc")
                    nc.scalar.activation(tc_t[:], ch_t[e][:, 0, :], AF.Tanh)
                    nc.gpsimd.tensor_mul(ch_t[e][:, 1, :], tc_t[:], a[:, 1024:1536])

                # S4: z = x + s for tile t (last on DVE queue: waits its DMA)
                if t < T:
                    z_t[t] = pz.tile([128, H], BF16, tag="z", name=f"z{t}")
                    nc.vector.tensor_add(
                        z_t[t][:], xs_t[t][:, 0, :], xs_t[t][:, 1, :]
                    )

                # S5: transpose z(t-2): zt[p, c, b] = z[b, c*128 + p]
                if 2 <= t < T + 2:
                    w = t - 2
                    zt_t[w] = pz.tile([128, 4, 128], BF16, tag="zt", name=f"zt{w}")
                    nc.sync.dma_start_transpose(zt_t[w][:], z_t[w][:])

                # S6: write out tile t-7 (c|h long complete)
                if t >= 7:
                    o = t - 7
                    orows = slice(o * 128, (o + 1) * 128)
                    nc.sync.dma_start(out=out[orows, :, :], in_=ch_t[o][:])

    nc.compile()
    return nc


def pack_inputs(inputs, short_term_memory, Wf, bf, Wi, bi, Wg, bg, Wo, bo):
    import ml_dtypes

    bf16 = ml_dtypes.bfloat16
    x = np.asarray(inputs, np.float32).astype(bf16)
    s = np.asarray(short_term_memory, np.float32).astype(bf16)
    xs = np.ascontiguousarray(np.stack([x, s], axis=1))  # [B, 2, H]
    Ws = [Wf, Wi, Wo, Wg]
    bs = [bf, bi, bo, bg]
    wt = np.empty((128, 4, 2048), bf16)
    for gs, W in enumerate(Ws):
        Wt = np.ascontiguousarray(np.asarray(W, np.float32).T)  # [h, j] = W[j, h]
        # wt[p, kc, gs*512 + j] = W[j, kc*128 + p]
        wt[:, :, gs * 512 : (gs + 1) * 512] = (
            Wt.reshape(4, 128, 512).transpose(1, 0, 2).astype(bf16)
        )
    bias = np.empty((128, 2048), np.float32)
    for gs, b in enumerate(bs):
        bias[:, gs * 512 : (gs + 1) * 512] = np.asarray(b, np.float32)[None, :]
    return {"xs": xs, "wt": wt, "bias": bias}


class Runner:
    """Compiles the module once and keeps a reusable jitted executor."""

    def __init__(self, nc=None, n_cores=N_CORES):
        import jax
        from concourse import bass2jax as b2j

        self.jax = jax
        self.n_cores = n_cores
        self.nc = nc or build_module(n_cores=n_cores)
        b2j.install_neuronx_cc_hook()

        # dump the final (renamed) NEFF so neuron-profile can pair it with NTFFs
        if not getattr(b2j, "_neff_dump_patched", False):
            orig = b2j.rename_neff_tensors_and_patch_header

            def _patched(neff_path, mapping):
                data = orig(neff_path, mapping)
                with open(NEFF_DUMP, "wb") as f:
                    f.write(data)
                return data

            b2j.rename_neff_tensors_and_patch_header = _patched
            b2j._neff_dump_patched = True

        from jax.experimental.shard_map import shard_map
        from jax.sharding import Mesh, NamedSharding, PartitionSpec

        part_name = (
            self.nc.partition_id_tensor.name if self.nc.partition_id_tensor else None
        )
        in_names, out_names, out_avals = [], [], []
        self.out_shapes = {}
        for alloc in self.nc.m.functions[0].allocations:
            if not isinstance(alloc, mybir.MemoryLocationSet):
                continue
            name = alloc.memorylocations[0].name
            if alloc.kind == "ExternalInput":
                if name != part_name:
                    in_names.append(name)
            elif alloc.kind == "ExternalOutput":
                out_names.append(name)
                shape = tuple(alloc.tensor_shape)
                dt = mybir.dt.np(alloc.dtype)
                out_avals.append(jax.core.ShapedArray(shape, dt))
                self.out_shapes[name] = (shape, dt)
        self.in_names, self.out_names = in_names, out_names
        nc_ref = self.nc

        bind_names = list(in_names) + list(out_names)
        if part_name is not None:
            bind_names.append(part_name)

        def _body(*args):
            operands = list(args)
            if part_name is not None:
                operands.append(b2j.partition_id_tensor())
            outs = b2j._bass_exec_p.bind(
                *operands,
                out_avals=tuple(out_avals),
                in_names=tuple(bind_names),
                out_names=tuple(out_names),
                lowering_input_output_aliases=(),
                sim_require_finite=False,
                sim_require_nnan=False,
                nc=nc_ref,
            )
            return tuple(outs)

        devices = jax.devices()[: self.n_cores]
        mesh = Mesh(np.asarray(devices), ("core",))
        spec = PartitionSpec("core")
        n_args = len(in_names) + len(out_names)
        self.sharding = NamedSharding(mesh, spec)
        self.fn = jax.jit(
            shard_map(
                _body,
                mesh=mesh,
                in_specs=(spec,) * n_args,
                out_specs=(spec,) * len(out_names),
                check_rep=False,
            ),
            keep_unused=True,
        )
        self._dev_args = None

    def stage(self, packed):
        """Transfer inputs (sharded/replicated as needed) to devices once."""
        jax = self.jax
        nc_n = self.n_cores
        args = []
        for name in self.in_names:
            a = packed[name]
            if name == "xs":
                glob = a  # [B, 2, H]; axis-0 shard = per-core [B_CORE, 2, H]
            else:
                glob = np.concatenate([a] * nc_n, axis=0)  # replicate
            args.append(glob)
        for name in self.out_names:
            shape, dt = self.out_shapes[name]
            args.append(np.zeros((shape[0] * nc_n,) + shape[1:], dt))
        self._dev_args = [jax.device_put(a, self.sharding) for a in args]

    def execute(self):
        outs = self.fn(*self._dev_args)
        self.jax.block_until_ready(outs)
        return outs

    def run(self, packed):
        self.stage(packed)
        outs = self.execute()
        res = {}
        for name, arr in zip(self.out_names, outs):
            a = np.asarray(arr)  # [n_cores*d0, ...]
            shape, _ = self.out_shapes[name]
            res[name] = a.reshape((self.n_cores, shape[0]) + tuple(shape[1:]))
        return res


_RUNNER = None


def _get_runner():
    global _RUNNER
    if _RUNNER is None:
        _RUNNER = Runner()
    return _RUNNER


def kernel(**inputs):
    r = _get_runner()
    packed = pack_inputs(**inputs)
    res = r.run(packed)
    per_core = res["out"]  # [8, 8192, 2, 512] bf16
    return np.ascontiguousarray(
        per_core.transpose(2, 0, 1, 3).reshape(2, B, H).astype(np.float32)
    )


if __name__ == "__main__":
    nc = build_module()
    print("module built + compiled OK")


# revision 8
# speedup vs baseline: 1.1384x; 1.0669x over previous
"""LSTMCell (B=65536, H=512) Bass/Tile kernel for 8 trn2 NeuronCores.

Data-parallel over batch: each core processes 8192 rows (64 tiles of 128).
Inputs are staged to DRAM as bf16 (halves HBM-in traffic); outputs written
bf16 and upcast to fp32 on host (halves HBM-out traffic). Per 128-row tile:
  z = x + stm                      (DVE, bf16)
  zT via XBAR DMA transpose        (DMA ucode, no PE/PSUM involvement)
  gates = zT.T @ W (bf16)          (TensorE, 4 matmuls of 2048 cols, PSUM x2)
  pre = gates + bias               (DVE, one [128,2048] op, drains PSUM)
  sigmoid(f,i,o) / tanh(g) -> bf16 (ACT, two ops)
  prod = si*tg                     (GpSimd)
  c = sf + prod                    (DVE)
  tanh(c)                          (ACT)
  h = tc*so                        (GpSimd)
Engine budget/tile ~ PE 3.4us > ACT 3.0 > DVE 2.9 > GpSimd 2.2 > Sync 2.5.
"""

import os
import sys

if "/opt/trn_rl_repo" not in sys.path:
    sys.path.insert(0, "/opt/trn_rl_repo")

import numpy as np

import concourse.bacc as bacc
import concourse.mybir as mybir
import concourse.tile as tile

N_CORES = 8
B, H = 65536, 512
B_CORE = B // N_CORES  # 8192
F32 = mybir.dt.float32
BF16 = mybir.dt.bfloat16
AF = mybir.ActivationFunctionType

NEFF_DUMP = "/tmp/lstm_kernel.neff"

# gate order in the packed weight/bias/psum layout: sigmoid gates first so one
# ACT op covers [0:1536], tanh gate last at [1536:2048]
#   slot 0: f (sigmoid), 1: i (sigmoid), 2: o (sigmoid), 3: g (tanh)


def build_module(b_core=B_CORE, n_cores=N_CORES):
    nc = bacc.Bacc(
        "TRN2",
        target_bir_lowering=False,
        debug=False,
        num_devices=n_cores,
    )
    xs = nc.dram_tensor("xs", [b_core, 2, H], BF16, kind="ExternalInput").ap()
    wt = nc.dram_tensor("wt", [128, 4, 2048], BF16, kind="ExternalInput").ap()
    bias = nc.dram_tensor("bias", [128, 2048], F32, kind="ExternalInput").ap()
    out = nc.dram_tensor("out", [b_core, 2, H], BF16, kind="ExternalOutput").ap()

    T = b_core // 128

    # Software pipeline with stage offsets so every in-order engine queue only
    # sees instructions whose data is (nearly) ready — no head-of-line stalls:
    #   iter t:  Sync: in(t), xbar(t-2), out(t-7)
    #            PE:   matmuls(t-4)
    #            DVE:  evac(t-4), c-add(t-5), z-add(t)
    #            ACT:  sig(t-4), tanhg(t-4), tanhc(t-5)
    #            Gp:   prod(t-4), h-mul(t-5)
    with tile.TileContext(nc) as tc:
        with (
            tc.tile_pool(name="const", bufs=1) as cpool,
            tc.tile_pool(name="pin", bufs=8) as pin,
            tc.tile_pool(name="pz", bufs=10) as pz,
            tc.tile_pool(name="pmid", bufs=6) as pmid,
            tc.tile_pool(name="pout", bufs=8) as pout,
            tc.tile_pool(name="pgates", bufs=2, space="PSUM") as pg,
        ):
            wt_sb = cpool.tile([128, 4, 2048], BF16)
            nc.sync.dma_start(out=wt_sb[:], in_=wt[:])
            bias_sb = cpool.tile([128, 2048], F32)
            nc.sync.dma_start(out=bias_sb[:], in_=bias[:])

            xs_t = [None] * T
            z_t = [None] * T
            zt_t = [None] * T
            acts_t = [None] * T
            prod_t = [None] * T
            ch_t = [None] * T

            for t in range(T + 7):
                # S0: input prefetch (x|s row-interleaved, one DMA)
                if t < T:
                    rows = slice(t * 128, (t + 1) * 128)
                    xs_t[t] = pin.tile([128, 2, H], BF16, tag="xs", name=f"xs{t}")
                    nc.sync.dma_start(out=xs_t[t][:], in_=xs[rows, :, :])

                # S1: matmuls for tile t-4 (psum bank per 512-col gate chunk)
                if 4 <= t < T + 4:
                    m = t - 4
                    g_ps = pg.tile([128, 2048], F32, tag="gates")
                    for k in range(4):
                        for gs in range(4):
                            nc.tensor.matmul(
                                g_ps[:, gs * 512 : (gs + 1) * 512],
                                zt_t[m][:, k, :],
                                wt_sb[:, k, gs * 512 : (gs + 1) * 512],
                                start=(k == 0),
                                stop=(k == 3),
                            )
                    # S2: drain psum (bias add), activations, input-gate product
                    pre = pmid.tile([128, 2048], BF16, tag="pre")
                    nc.vector.tensor_add(pre[:], g_ps[:], bias_sb[:])
                    acts_t[m] = pmid.tile([128, 2048], BF16, tag="acts", name=f"acts{m}")
                    nc.scalar.activation(
                        acts_t[m][:, 0:1536], pre[:, 0:1536], AF.Sigmoid
                    )
                    nc.scalar.activation(
                        acts_t[m][:, 1536:2048], pre[:, 1536:2048], AF.Tanh
                    )
                    prod_t[m] = pmid.tile([128, H], BF16, tag="prod", name=f"prod{m}")
                    nc.gpsimd.tensor_mul(
                        prod_t[m][:], acts_t[m][:, 512:1024], acts_t[m][:, 1536:2048]
                    )

                # S3: tail for tile t-5 (prod long ready -> no DVE queue stall)
                if 5 <= t < T + 5:
                    e = t - 5
                    a = acts_t[e]
                    ch_t[e] = pout.tile([128, 2, H], BF16, tag="ch", name=f"ch{e}")
                    nc.vector.tensor_add(ch_t[e][:, 0, :], a[:, 0:512], prod_t[e][:])
                    tc_t = pmid.tile([128, H], BF16, tag="tc")
                    nc.scalar.activation(tc_t[:], ch_t[e][:, 0, :], AF.Tanh)
                    nc.gpsimd.tensor_mul(ch_t[e][:, 1, :], tc_t[:], a[:, 1024:1536])

                # S4: z = x + s for tile t (last on DVE queue: waits its DMA)
                if t < T:
                    z_t[t] = pz.tile([128, H], BF16, tag="z", name=f"z{t}")
                    nc.vector.tensor_add(
                        z_t[t][:], xs_t[t][:, 0, :], xs_t[t][:, 1, :]
                    )

                # S5: transpose z(t-2): zt[p, c, b] = z[b, c*128 + p]
                if 2 <= t < T + 2:
                    w = t - 2
                    zt_t[w] = pz.tile([128, 4, 128], BF16, tag="zt", name=f"zt{w}")
                    nc.sync.dma_start_transpose(zt_t[w][:], z_t[w][:])

                # S6: write out tile t-7 (c|h long complete)
                if t >= 7:
                    o = t - 7
                    orows = slice(o * 128, (o + 1) * 128)
                    nc.sync.dma_start(out=out[orows, :, :], in_=ch_t[o][:])

    nc.compile()
    return nc


def pack_inputs(inputs, short_term_memory, Wf, bf, Wi, bi, Wg, bg, Wo, bo):
    import ml_dtypes

    bf16 = ml_dtypes.bfloat16
    x = np.asarray(inputs, np.float32).astype(bf16)
    s = np.asarray(short_term_memory, np.float32).astype(bf16)
    xs = np.ascontiguousarray(np.stack([x, s], axis=1))  # [B, 2, H]
    Ws = [Wf, Wi, Wo, Wg]
    bs = [bf, bi, bo, bg]
    wt = np.empty((128, 4, 2048), bf16)
    for gs, W in enumerate(Ws):
        Wt = np.ascontiguousarray(np.asarray(W, np.float32).T)  # [h, j] = W[j, h]
        # wt[p, kc, gs*512 + j] = W[j, kc*128 + p]
        wt[:, :, gs * 512 : (gs + 1) * 512] = (
            Wt.reshape(4, 128, 512).transpose(1, 0, 2).astype(bf16)
        )
    bias = np.empty((128, 2048), np.float32)
    for gs, b in enumerate(bs):
        bias[:, gs * 512 : (gs + 1) * 512] = np.asarray(b, np.float32)[None, :]
    return {"xs": xs, "wt": wt, "bias": bias}


class Runner:
    """Compiles the module once and keeps a reusable jitted executor."""

    def __init__(self, nc=None, n_cores=N_CORES):
        import jax
        from concourse import bass2jax as b2j

        self.jax = jax
        self.n_cores = n_cores
        self.nc = nc or build_module(n_cores=n_cores)
        b2j.install_neuronx_cc_hook()

        # dump the final (renamed) NEFF so neuron-profile can pair it with NTFFs
        if not getattr(b2j, "_neff_dump_patched", False):
            orig = b2j.rename_neff_tensors_and_patch_header

            def _patched(neff_path, mapping):
                data = orig(neff_path, mapping)
                with open(NEFF_DUMP, "wb") as f:
                    f.write(data)
                return data

            b2j.rename_neff_tensors_and_patch_header = _patched
            b2j._neff_dump_patched = True

        from jax.experimental.shard_map import shard_map
        from jax.sharding import Mesh, NamedSharding, PartitionSpec

        part_name = (
            self.nc.partition_id_tensor.name if self.nc.partition_id_tensor else None
        )
        in_names, out_names, out_avals = [], [], []
        self.out_shapes = {}
        for alloc in self.nc.m.functions[0].allocations:
            if not isinstance(alloc, mybir.MemoryLocationSet):
                continue
            name = alloc.memorylocations[0].name
            if alloc.kind == "ExternalInput":
                if name != part_name:
                    in_names.append(name)
            elif alloc.kind == "ExternalOutput":
                out_names.append(name)
                shape = tuple(alloc.tensor_shape)
                dt = mybir.dt.np(alloc.dtype)
                out_avals.append(jax.core.ShapedArray(shape, dt))
                self.out_shapes[name] = (shape, dt)
        self.in_names, self.out_names = in_names, out_names
        nc_ref = self.nc

        bind_names = list(in_names) + list(out_names)
        if part_name is not None:
            bind_names.append(part_name)

        def _body(*args):
            operands = list(args)
            if part_name is not None:
                operands.append(b2j.partition_id_tensor())
            outs = b2j._bass_exec_p.bind(
                *operands,
                out_avals=tuple(out_avals),
                in_names=tuple(bind_names),
                out_names=tuple(out_names),
                lowering_input_output_aliases=(),
                sim_require_finite=False,
                sim_require_nnan=False,
                nc=nc_ref,
            )
            return tuple(outs)

        devices = jax.devices()[: self.n_cores]
        mesh = Mesh(np.asarray(devices), ("core",))
        spec = PartitionSpec("core")
        n_args = len(in_names) + len(out_names)
        self.sharding = NamedSharding(mesh, spec)
        self.fn = jax.jit(
            shard_map(
                _body,
                mesh=mesh,
                in_specs=(spec,) * n_args,
                out_specs=(spec,) * len(out_names),
                check_rep=False,
            ),
            keep_unused=True,
        )
        self._dev_args = None

    def stage(self, packed):
        """Transfer inputs (sharded/replicated as needed) to devices once."""
        jax = self.jax
        nc_n = self.n_cores
        args = []
        for name in self.in_names:
            a = packed[name]
            if name == "xs":
                glob = a  # [B, 2, H]; axis-0 shard = per-core [B_CORE, 2, H]
            else:
                glob = np.concatenate([a] * nc_n, axis=0)  # replicate
            args.append(glob)
        for name in self.out_names:
            shape, dt = self.out_shapes[name]
            args.append(np.zeros((shape[0] * nc_n,) + shape[1:], dt))
        self._dev_args = [jax.device_put(a, self.sharding) for a in args]

    def execute(self):
        outs = self.fn(*self._dev_args)
        self.jax.block_until_ready(outs)
        return outs

    def run(self, packed):
        self.stage(packed)
        outs = self.execute()
        res = {}
        for name, arr in zip(self.out_names, outs):
            a = np.asarray(arr)  # [n_cores*d0, ...]
            shape, _ = self.out_shapes[name]
            res[name] = a.reshape((self.n_cores, shape[0]) + tuple(shape[1:]))
        return res


_RUNNER = None


def _get_runner():
    global _RUNNER
    if _RUNNER is None:
        _RUNNER = Runner()
    return _RUNNER


def kernel(**inputs):
    r = _get_runner()
    packed = pack_inputs(**inputs)
    res = r.run(packed)
    per_core = res["out"]  # [8, 8192, 2, 512] bf16
    return np.ascontiguousarray(
        per_core.transpose(2, 0, 1, 3).reshape(2, B, H).astype(np.float32)
    )


if __name__ == "__main__":
    nc = build_module()
    print("module built + compiled OK")


# revision 12
# speedup vs baseline: 1.4529x; 1.2763x over previous
"""LSTMCell (B=65536, H=512) Bass/Tile kernel for 8 trn2 NeuronCores.

Data-parallel over batch: each core processes 8192 rows (64 tiles of 128).
Inputs are staged to DRAM as bf16 (halves HBM-in traffic); outputs written
bf16 and upcast to fp32 on host (halves HBM-out traffic). Per 128-row tile:
  z = x + stm                      (DVE, bf16)
  zT via XBAR DMA transpose        (DMA ucode, no PE/PSUM involvement)
  gates = zT.T @ W (bf16)          (TensorE, 4 matmuls of 2048 cols, PSUM x2)
  pre = gates + bias               (DVE, one [128,2048] op, drains PSUM)
  sigmoid(f,i,o) / tanh(g) -> bf16 (ACT, two ops)
  prod = si*tg                     (GpSimd)
  c = sf + prod                    (DVE)
  tanh(c)                          (ACT)
  h = tc*so                        (GpSimd)
Engine budget/tile ~ PE 3.4us > ACT 3.0 > DVE 2.9 > GpSimd 2.2 > Sync 2.5.
"""

import os
import sys

if "/opt/trn_rl_repo" not in sys.path:
    sys.path.insert(0, "/opt/trn_rl_repo")

import numpy as np

import concourse.bacc as bacc
import concourse.mybir as mybir
import concourse.tile as tile

N_CORES = 8
B, H = 65536, 512
B_CORE = B // N_CORES  # 8192
F32 = mybir.dt.float32
BF16 = mybir.dt.bfloat16
AF = mybir.ActivationFunctionType

NEFF_DUMP = "/tmp/lstm_kernel.neff"

# gate order in the packed weight/bias/psum layout: sigmoid gates first so one
# ACT op covers [0:1536], tanh gate last at [1536:2048]
#   slot 0: f (sigmoid), 1: i (sigmoid), 2: o (sigmoid), 3: g (tanh)


def build_module(b_core=B_CORE, n_cores=N_CORES):
    nc = bacc.Bacc(
        "TRN2",
        target_bir_lowering=False,
        debug=False,
        num_devices=n_cores,
    )
    x_d = nc.dram_tensor("x", [b_core, H], BF16, kind="ExternalInput").ap()
    s_d = nc.dram_tensor("s", [b_core, H], BF16, kind="ExternalInput").ap()
    wt = nc.dram_tensor("wt", [128, 4, 2048], BF16, kind="ExternalInput").ap()
    bias = nc.dram_tensor("bias", [128, 2048], F32, kind="ExternalInput").ap()
    out = nc.dram_tensor("out", [b_core, 2, H], BF16, kind="ExternalOutput").ap()

    T = b_core // 128  # 64 tiles
    NQ = T // 4  # 16 quads of 512 rows; xbar-transposed straight from DRAM
    PF = 2  # quad prefetch distance

    # Software pipeline; per-engine queues only see nearly-ready work:
    #   iter t:  Sync: [xbar x/s for quad t//4+PF], out(t-5)
    #            DVE:  [ztadd(quad t//4)], evacA(t-2), evacB(t-2), c-add(t-3)
    #            PE:   matmuls halfA(t-2) then halfB(t-2)
    #            ACT:  sig(t-2), tanhg(t-2), tanhc(t-3)
    #            Gp:   prod(t-2), h-mul(t-3)
    # PSUM: two half tiles [128,1024] (2 banks) x bufs=2 per half = 8 banks.
    # Each half's evac completes during the other half's matmuls -> PE never
    # waits on a drain even with one-slot-late WAR semantics.
    with tile.TileContext(nc) as tc:
        with (
            tc.tile_pool(name="const", bufs=1) as cpool,
            tc.tile_pool(name="pxt", bufs=4) as pxt,
            tc.tile_pool(name="pzt", bufs=3) as pzt,
            tc.tile_pool(name="pmid", bufs=4) as pmid,
            tc.tile_pool(name="pout", bufs=6) as pout,
            tc.tile_pool(name="pgates", bufs=2, space="PSUM") as pg,
        ):
            wt_sb = cpool.tile([128, 4, 2048], BF16)
            nc.sync.dma_start(out=wt_sb[:], in_=wt[:])
            bias_sb = cpool.tile([128, 2048], F32)
            nc.sync.dma_start(out=bias_sb[:], in_=bias[:])

            xt_q = [None] * NQ
            st_q = [None] * NQ
            zt_q = [None] * NQ
            acts_t = [None] * T
            prod_t = [None] * T
            ch_t = [None] * T

            def xbar_quad(q):
                # xt[p, c, b] = x[q*512 + b, c*128 + p]
                qrows = slice(q * 512, (q + 1) * 512)
                xt_q[q] = pxt.tile([128, 4, 512], BF16, tag="xt", name=f"xt{q}")
                nc.sync.dma_start_transpose(xt_q[q][:], x_d[qrows, :])
                st_q[q] = pxt.tile([128, 4, 512], BF16, tag="st", name=f"st{q}")
                nc.sync.dma_start_transpose(st_q[q][:], s_d[qrows, :])

            for q in range(min(PF, NQ)):
                xbar_quad(q)

            for t in range(T + 5):
                # S0: prefetch transposed x/s for quad t//4+PF (no upstream deps)
                if t < T and t % 4 == 0 and t // 4 + PF < NQ:
                    xbar_quad(t // 4 + PF)

                # S1: zT = xT + sT for quad t//4 (one [128,2048] bf16 2x op)
                if t < T and t % 4 == 0:
                    qa = t // 4
                    zt_q[qa] = pzt.tile([128, 4, 512], BF16, tag="zt", name=f"zt{qa}")
                    nc.vector.tensor_add(zt_q[qa][:], xt_q[qa][:], st_q[qa][:])

                # S2: matmuls + psum drain + activations for tile t-2
                if 2 <= t < T + 2:
                    m = t - 2
                    zt = zt_q[m // 4]
                    boff = (m % 4) * 128  # batch offset within the quad
                    pre = pmid.tile([128, 2048], BF16, tag="pre")
                    acts_t[m] = pmid.tile(
                        [128, 2048], BF16, tag="acts", name=f"acts{m}"
                    )
                    for half in range(2):
                        g_ps = pg.tile(
                            [128, 1024], F32, tag=f"g{half}", name=f"g{half}_{m}"
                        )
                        for k in range(4):
                            for g in range(2):
                                gs = half * 2 + g
                                nc.tensor.matmul(
                                    g_ps[:, g * 512 : (g + 1) * 512],
                                    zt[:, k, boff : boff + 128],
                                    wt_sb[:, k, gs * 512 : (gs + 1) * 512],
                                    start=(k == 0),
                                    stop=(k == 3),
                                )
                        cols = slice(half * 1024, (half + 1) * 1024)
                        nc.vector.tensor_add(pre[:, cols], g_ps[:], bias_sb[:, cols])
                    nc.scalar.activation(
                        acts_t[m][:, 0:1536], pre[:, 0:1536], AF.Sigmoid
                    )
                    nc.scalar.activation(
                        acts_t[m][:, 1536:2048], pre[:, 1536:2048], AF.Tanh
                    )
                    prod_t[m] = pmid.tile([128, H], BF16, tag="prod", name=f"prod{m}")
                    nc.gpsimd.tensor_mul(
                        prod_t[m][:], acts_t[m][:, 512:1024], acts_t[m][:, 1536:2048]
                    )

                # S3: tail for tile t-3 (prod long ready)
                if 3 <= t < T + 3:
                    e = t - 3
                    a = acts_t[e]
                    ch_t[e] = pout.tile([128, 2, H], BF16, tag="ch", name=f"ch{e}")
                    nc.vector.tensor_add(ch_t[e][:, 0, :], a[:, 0:512], prod_t[e][:])
                    tc_t = pmid.tile([128, H], BF16, tag="tc")
                    nc.scalar.activation(tc_t[:], ch_t[e][:, 0, :], AF.Tanh)
                    nc.gpsimd.tensor_mul(ch_t[e][:, 1, :], tc_t[:], a[:, 1024:1536])

                # S4: write out tile t-5
                if t >= 5:
                    o = t - 5
                    orows = slice(o * 128, (o + 1) * 128)
                    nc.sync.dma_start(out=out[orows, :, :], in_=ch_t[o][:])

    nc.compile()
    return nc


def pack_inputs(inputs, short_term_memory, Wf, bf, Wi, bi, Wg, bg, Wo, bo):
    import ml_dtypes

    bf16 = ml_dtypes.bfloat16
    x = np.asarray(inputs, np.float32).astype(bf16)
    s = np.asarray(short_term_memory, np.float32).astype(bf16)
    Ws = [Wf, Wi, Wo, Wg]
    bs = [bf, bi, bo, bg]
    wt = np.empty((128, 4, 2048), bf16)
    for gs, W in enumerate(Ws):
        Wt = np.ascontiguousarray(np.asarray(W, np.float32).T)  # [h, j] = W[j, h]
        # wt[p, kc, gs*512 + j] = W[j, kc*128 + p]
        wt[:, :, gs * 512 : (gs + 1) * 512] = (
            Wt.reshape(4, 128, 512).transpose(1, 0, 2).astype(bf16)
        )
    bias = np.empty((128, 2048), np.float32)
    for gs, b in enumerate(bs):
        bias[:, gs * 512 : (gs + 1) * 512] = np.asarray(b, np.float32)[None, :]
    return {"x": x, "s": s, "wt": wt, "bias": bias}


class Runner:
    """Compiles the module once and keeps a reusable jitted executor."""

    def __init__(self, nc=None, n_cores=N_CORES):
        import jax
        from concourse import bass2jax as b2j

        self.jax = jax
        self.n_cores = n_cores
        self.nc = nc or build_module(n_cores=n_cores)
        b2j.install_neuronx_cc_hook()

        # dump the final (renamed) NEFF so neuron-profile can pair it with NTFFs
        if not getattr(b2j, "_neff_dump_patched", False):
            orig = b2j.rename_neff_tensors_and_patch_header

            def _patched(neff_path, mapping):
                data = orig(neff_path, mapping)
                with open(NEFF_DUMP, "wb") as f:
                    f.write(data)
                return data

            b2j.rename_neff_tensors_and_patch_header = _patched
            b2j._neff_dump_patched = True

        from jax.experimental.shard_map import shard_map
        from jax.sharding import Mesh, NamedSharding, PartitionSpec

        part_name = (
            self.nc.partition_id_tensor.name if self.nc.partition_id_tensor else None
        )
        in_names, out_names, out_avals = [], [], []
        self.out_shapes = {}
        for alloc in self.nc.m.functions[0].allocations:
            if not isinstance(alloc, mybir.MemoryLocationSet):
                continue
            name = alloc.memorylocations[0].name
            if alloc.kind == "ExternalInput":
                if name != part_name:
                    in_names.append(name)
            elif alloc.kind == "ExternalOutput":
                out_names.append(name)
                shape = tuple(alloc.tensor_shape)
                dt = mybir.dt.np(alloc.dtype)
                out_avals.append(jax.core.ShapedArray(shape, dt))
                self.out_shapes[name] = (shape, dt)
        self.in_names, self.out_names = in_names, out_names
        nc_ref = self.nc

        bind_names = list(in_names) + list(out_names)
        if part_name is not None:
            bind_names.append(part_name)

        def _body(*args):
            operands = list(args)
            if part_name is not None:
                operands.append(b2j.partition_id_tensor())
            outs = b2j._bass_exec_p.bind(
                *operands,
                out_avals=tuple(out_avals),
                in_names=tuple(bind_names),
                out_names=tuple(out_names),
                lowering_input_output_aliases=(),
                sim_require_finite=False,
                sim_require_nnan=False,
                nc=nc_ref,
            )
            return tuple(outs)

        devices = jax.devices()[: self.n_cores]
        mesh = Mesh(np.asarray(devices), ("core",))
        spec = PartitionSpec("core")
        n_args = len(in_names) + len(out_names)
        self.sharding = NamedSharding(mesh, spec)
        self.fn = jax.jit(
            shard_map(
                _body,
                mesh=mesh,
                in_specs=(spec,) * n_args,
                out_specs=(spec,) * len(out_names),
                check_rep=False,
            ),
            keep_unused=True,
        )
        self._dev_args = None

    def stage(self, packed):
        """Transfer inputs (sharded/replicated as needed) to devices once."""
        jax = self.jax
        nc_n = self.n_cores
        args = []
        for name in self.in_names:
            a = packed[name]
            if name in ("x", "s"):
                glob = a  # [B, H]; axis-0 shard = per-core [B_CORE, H]
            else:
                glob = np.concatenate([a] * nc_n, axis=0)  # replicate
            args.append(glob)
        for name in self.out_names:
            shape, dt = self.out_shapes[name]
            args.append(np.zeros((shape[0] * nc_n,) + shape[1:], dt))
        self._dev_args = [jax.device_put(a, self.sharding) for a in args]

    def execute(self):
        outs = self.fn(*self._dev_args)
        self.jax.block_until_ready(outs)
        return outs

    def run(self, packed):
        self.stage(packed)
        outs = self.execute()
        res = {}
        for name, arr in zip(self.out_names, outs):
            a = np.asarray(arr)  # [n_cores*d0, ...]
            shape, _ = self.out_shapes[name]
            res[name] = a.reshape((self.n_cores, shape[0]) + tuple(shape[1:]))
        return res


_RUNNER = None


def _get_runner():
    global _RUNNER
    if _RUNNER is None:
        _RUNNER = Runner()
    return _RUNNER


def kernel(**inputs):
    r = _get_runner()
    packed = pack_inputs(**inputs)
    res = r.run(packed)
    per_core = res["out"]  # [8, 8192, 2, 512] bf16
    return np.ascontiguousarray(
        per_core.transpose(2, 0, 1, 3).reshape(2, B, H).astype(np.float32)
    )


if __name__ == "__main__":
    nc = build_module()
    print("module built + compiled OK")


# revision 15
# speedup vs baseline: 1.5892x; 1.0938x over previous
"""LSTMCell (B=65536, H=512) Bass/Tile kernel for 8 trn2 NeuronCores.

Data-parallel over batch: each core processes 8192 rows (64 tiles of 128).
Inputs are staged to DRAM as bf16 (halves HBM-in traffic); outputs written
bf16 and upcast to fp32 on host (halves HBM-out traffic). Per 128-row tile:
  z = x + stm                      (DVE, bf16)
  zT via XBAR DMA transpose        (DMA ucode, no PE/PSUM involvement)
  gates = zT.T @ W (bf16)          (TensorE, 4 matmuls of 2048 cols, PSUM x2)
  pre = gates + bias               (DVE, one [128,2048] op, drains PSUM)
  sigmoid(f,i,o) / tanh(g) -> bf16 (ACT, two ops)
  prod = si*tg                     (GpSimd)
  c = sf + prod                    (DVE)
  tanh(c)                          (ACT)
  h = tc*so                        (GpSimd)
Engine budget/tile ~ PE 3.4us > ACT 3.0 > DVE 2.9 > GpSimd 2.2 > Sync 2.5.
"""

import os
import sys

if "/opt/trn_rl_repo" not in sys.path:
    sys.path.insert(0, "/opt/trn_rl_repo")

import numpy as np

import concourse.bacc as bacc
import concourse.mybir as mybir
import concourse.tile as tile

N_CORES = 8
B, H = 65536, 512
B_CORE = B // N_CORES  # 8192
F32 = mybir.dt.float32
BF16 = mybir.dt.bfloat16
AF = mybir.ActivationFunctionType

NEFF_DUMP = "/tmp/lstm_kernel.neff"

# gate order in the packed weight/bias/psum layout: sigmoid gates first so one
# ACT op covers [0:1536], tanh gate last at [1536:2048]
#   slot 0: f (sigmoid), 1: i (sigmoid), 2: o (sigmoid), 3: g (tanh)



def _enable_ldw_opt():
    """Rewrite --enable-ldw-opt=false -> true in the walrus compile argv."""
    from concourse import bass_utils as _bu

    if getattr(_bu, "_ldw_opt_patched", False):
        return
    _orig = _bu.run_command

    def _patched(cmd, *a, **kw):
        cmd = [
            c.replace("--enable-ldw-opt=false", "--enable-ldw-opt=false")
            if isinstance(c, str)
            else c
            for c in cmd
        ]
        return _orig(cmd, *a, **kw)

    _bu.run_command = _patched
    _bu._ldw_opt_patched = True


def build_module(b_core=B_CORE, n_cores=N_CORES):
    _enable_ldw_opt()
    nc = bacc.Bacc(
        "TRN2",
        target_bir_lowering=False,
        debug=False,
        num_devices=n_cores,
    )
    x_d = nc.dram_tensor("x", [b_core, H], BF16, kind="ExternalInput").ap()
    s_d = nc.dram_tensor("s", [b_core, H], BF16, kind="ExternalInput").ap()
    wt = nc.dram_tensor("wt", [128, 4, 2048], BF16, kind="ExternalInput").ap()
    bias = nc.dram_tensor("bias", [128, 2048], F32, kind="ExternalInput").ap()
    out = nc.dram_tensor("out", [b_core, 2, H], BF16, kind="ExternalOutput").ap()

    T = b_core // 128  # 64 tiles
    NQ = T // 4  # 16 quads of 512 rows; xbar-transposed straight from DRAM
    PF = 3  # quad prefetch distance

    # Software pipeline; per-engine queues only see nearly-ready work:
    #   iter t:  Sync: [xbar x/s for quad t//4+PF], out(t-5)
    #            DVE:  [ztadd(quad t//4)], evacA(t-2), evacB(t-2), c-add(t-3)
    #            PE:   matmuls halfA(t-2) then halfB(t-2)
    #            ACT:  sig(t-2), tanhg(t-2), tanhc(t-3)
    #            Gp:   prod(t-2), h-mul(t-3)
    # PSUM: two half tiles [128,1024] (2 banks) x bufs=2 per half = 8 banks.
    # Each half's evac completes during the other half's matmuls -> PE never
    # waits on a drain even with one-slot-late WAR semantics.
    with tile.TileContext(nc) as tc:
        with (
            tc.tile_pool(name="const", bufs=1) as cpool,
            tc.tile_pool(name="pxt", bufs=5) as pxt,
            tc.tile_pool(name="pzt", bufs=3) as pzt,
            tc.tile_pool(name="pmid", bufs=4) as pmid,
            tc.tile_pool(name="pout", bufs=6) as pout,
            tc.tile_pool(name="pgates", bufs=2, space="PSUM") as pg,
        ):
            wt_sb = cpool.tile([128, 4, 2048], BF16)
            nc.sync.dma_start(out=wt_sb[:], in_=wt[:])
            bias_sb = cpool.tile([128, 2048], F32)
            nc.sync.dma_start(out=bias_sb[:], in_=bias[:])

            xt_q = [None] * NQ
            st_q = [None] * NQ
            zt_q = [None] * NQ
            acts_t = [None] * T
            prod_t = [None] * T
            ch_t = [None] * T

            def xbar_quad(q):
                # xt[p, c, b] = x[q*512 + b, c*128 + p]
                qrows = slice(q * 512, (q + 1) * 512)
                xt_q[q] = pxt.tile([128, 4, 512], BF16, tag="xt", name=f"xt{q}")
                nc.sync.dma_start_transpose(xt_q[q][:], x_d[qrows, :])
                st_q[q] = pxt.tile([128, 4, 512], BF16, tag="st", name=f"st{q}")
                nc.sync.dma_start_transpose(st_q[q][:], s_d[qrows, :])

            for q in range(min(PF, NQ)):
                xbar_quad(q)

            for t in range(T + 5):
                # S0: prefetch transposed x/s for quad t//4+PF (no upstream deps)
                if t < T and t % 4 == 0 and t // 4 + PF < NQ:
                    xbar_quad(t // 4 + PF)

                # S1: zT = xT + sT for quad t//4 (one [128,2048] bf16 2x op)
                if t < T and t % 4 == 0:
                    qa = t // 4
                    zt_q[qa] = pzt.tile([128, 4, 512], BF16, tag="zt", name=f"zt{qa}")
                    nc.vector.tensor_add(zt_q[qa][:], xt_q[qa][:], st_q[qa][:])

                # S2: matmuls + psum drain + activations for tile t-2
                if 2 <= t < T + 2:
                    m = t - 2
                    zt = zt_q[m // 4]
                    boff = (m % 4) * 128  # batch offset within the quad
                    pre = pmid.tile([128, 2048], BF16, tag="pre")
                    acts_t[m] = pmid.tile(
                        [128, 2048], BF16, tag="acts", name=f"acts{m}"
                    )
                    for half in range(2):
                        g_ps = pg.tile(
                            [128, 1024], F32, tag=f"g{half}", name=f"g{half}_{m}"
                        )
                        cols = slice(half * 1024, (half + 1) * 1024)
                        for k in range(4):
                            for g in range(2):
                                gs = half * 2 + g
                                nc.tensor.matmul(
                                    g_ps[:, g * 512 : (g + 1) * 512],
                                    zt[:, k, boff : boff + 128],
                                    wt_sb[:, k, gs * 512 : (gs + 1) * 512],
                                    start=(k == 0),
                                    stop=(k == 3),
                                )
                        nc.vector.tensor_add(pre[:, cols], g_ps[:], bias_sb[:, cols])
                    nc.scalar.activation(
                        acts_t[m][:, 0:1536], pre[:, 0:1536], AF.Sigmoid
                    )
                    nc.scalar.activation(
                        acts_t[m][:, 1536:2048], pre[:, 1536:2048], AF.Tanh
                    )
                    prod_t[m] = pmid.tile([128, H], BF16, tag="prod", name=f"prod{m}")
                    nc.gpsimd.tensor_mul(
                        prod_t[m][:], acts_t[m][:, 512:1024], acts_t[m][:, 1536:2048]
                    )

                # S3: tail for tile t-3 (prod long ready)
                if 3 <= t < T + 3:
                    e = t - 3
                    a = acts_t[e]
                    ch_t[e] = pout.tile([128, 2, H], BF16, tag="ch", name=f"ch{e}")
                    nc.vector.tensor_add(ch_t[e][:, 0, :], a[:, 0:512], prod_t[e][:])
                    tc_t = pmid.tile([128, H], BF16, tag="tc")
                    nc.scalar.activation(tc_t[:], ch_t[e][:, 0, :], AF.Tanh)
                    nc.gpsimd.tensor_mul(ch_t[e][:, 1, :], tc_t[:], a[:, 1024:1536])

                # S4: write out tile t-5
                if t >= 5:
                    o = t - 5
                    orows = slice(o * 128, (o + 1) * 128)
                    nc.sync.dma_start(out=out[orows, :, :], in_=ch_t[o][:])

    nc.compile()
    return nc


def pack_inputs(inputs, short_term_memory, Wf, bf, Wi, bi, Wg, bg, Wo, bo):
    import ml_dtypes

    bf16 = ml_dtypes.bfloat16
    x = np.asarray(inputs, np.float32).astype(bf16)
    s = np.asarray(short_term_memory, np.float32).astype(bf16)
    Ws = [Wf, Wi, Wo, Wg]
    bs = [bf, bi, bo, bg]
    wt = np.empty((128, 4, 2048), bf16)
    for gs, W in enumerate(Ws):
        Wt = np.ascontiguousarray(np.asarray(W, np.float32).T)  # [h, j] = W[j, h]
        # wt[p, kc, gs*512 + j] = W[j, kc*128 + p]
        wt[:, :, gs * 512 : (gs + 1) * 512] = (
            Wt.reshape(4, 128, 512).transpose(1, 0, 2).astype(bf16)
        )
    bias = np.empty((128, 2048), np.float32)
    for gs, b in enumerate(bs):
        bias[:, gs * 512 : (gs + 1) * 512] = np.asarray(b, np.float32)[None, :]
    return {"x": x, "s": s, "wt": wt, "bias": bias}


class Runner:
    """Compiles the module once and keeps a reusable jitted executor."""

    def __init__(self, nc=None, n_cores=N_CORES):
        import jax
        from concourse import bass2jax as b2j

        self.jax = jax
        self.n_cores = n_cores
        self.nc = nc or build_module(n_cores=n_cores)
        b2j.install_neuronx_cc_hook()

        # dump the final (renamed) NEFF so neuron-profile can pair it with NTFFs
        if not getattr(b2j, "_neff_dump_patched", False):
            orig = b2j.rename_neff_tensors_and_patch_header

            def _patched(neff_path, mapping):
                data = orig(neff_path, mapping)
                with open(NEFF_DUMP, "wb") as f:
                    f.write(data)
                return data

            b2j.rename_neff_tensors_and_patch_header = _patched
            b2j._neff_dump_patched = True

        from jax.experimental.shard_map import shard_map
        from jax.sharding import Mesh, NamedSharding, PartitionSpec

        part_name = (
            self.nc.partition_id_tensor.name if self.nc.partition_id_tensor else None
        )
        in_names, out_names, out_avals = [], [], []
        self.out_shapes = {}
        for alloc in self.nc.m.functions[0].allocations:
            if not isinstance(alloc, mybir.MemoryLocationSet):
                continue
            name = alloc.memorylocations[0].name
            if alloc.kind == "ExternalInput":
                if name != part_name:
                    in_names.append(name)
            elif alloc.kind == "ExternalOutput":
                out_names.append(name)
                shape = tuple(alloc.tensor_shape)
                dt = mybir.dt.np(alloc.dtype)
                out_avals.append(jax.core.ShapedArray(shape, dt))
                self.out_shapes[name] = (shape, dt)
        self.in_names, self.out_names = in_names, out_names
        nc_ref = self.nc

        bind_names = list(in_names) + list(out_names)
        if part_name is not None:
            bind_names.append(part_name)

        def _body(*args):
            operands = list(args)
            if part_name is not None:
                operands.append(b2j.partition_id_tensor())
            outs = b2j._bass_exec_p.bind(
                *operands,
                out_avals=tuple(out_avals),
                in_names=tuple(bind_names),
                out_names=tuple(out_names),
                lowering_input_output_aliases=(),
                sim_require_finite=False,
                sim_require_nnan=False,
                nc=nc_ref,
            )
            return tuple(outs)

        devices = jax.devices()[: self.n_cores]
        mesh = Mesh(np.asarray(devices), ("core",))
        spec = PartitionSpec("core")
        n_args = len(in_names) + len(out_names)
        self.sharding = NamedSharding(mesh, spec)
        self.fn = jax.jit(
            shard_map(
                _body,
                mesh=mesh,
                in_specs=(spec,) * n_args,
                out_specs=(spec,) * len(out_names),
                check_rep=False,
            ),
            keep_unused=True,
        )
        self._dev_args = None

    def stage(self, packed):
        """Transfer inputs (sharded/replicated as needed) to devices once."""
        jax = self.jax
        nc_n = self.n_cores
        args = []
        for name in self.in_names:
            a = packed[name]
            if name in ("x", "s"):
                glob = a  # [B, H]; axis-0 shard = per-core [B_CORE, H]
            else:
                glob = np.concatenate([a] * nc_n, axis=0)  # replicate
            args.append(glob)
        for name in self.out_names:
            shape, dt = self.out_shapes[name]
            args.append(np.zeros((shape[0] * nc_n,) + shape[1:], dt))
        self._dev_args = [jax.device_put(a, self.sharding) for a in args]

    def execute(self):
        outs = self.fn(*self._dev_args)
        self.jax.block_until_ready(outs)
        return outs

    def run(self, packed):
        self.stage(packed)
        outs = self.execute()
        res = {}
        for name, arr in zip(self.out_names, outs):
            a = np.asarray(arr)  # [n_cores*d0, ...]
            shape, _ = self.out_shapes[name]
            res[name] = a.reshape((self.n_cores, shape[0]) + tuple(shape[1:]))
        return res


_RUNNER = None


def _get_runner():
    global _RUNNER
    if _RUNNER is None:
        _RUNNER = Runner()
    return _RUNNER


def kernel(**inputs):
    r = _get_runner()
    packed = pack_inputs(**inputs)
    res = r.run(packed)
    per_core = res["out"]  # [8, 8192, 2, 512] bf16
    return np.ascontiguousarray(
        per_core.transpose(2, 0, 1, 3).reshape(2, B, H).astype(np.float32)
    )


if __name__ == "__main__":
    nc = build_module()
    print("module built + compiled OK")


# revision 16
# speedup vs baseline: 1.6056x; 1.0104x over previous
"""LSTMCell (B=65536, H=512) Bass/Tile kernel for 8 trn2 NeuronCores.

Data-parallel over batch: each core processes 8192 rows (64 tiles of 128).
Inputs are staged to DRAM as bf16 (halves HBM-in traffic); outputs written
bf16 and upcast to fp32 on host (halves HBM-out traffic). Per 128-row tile:
  z = x + stm                      (DVE, bf16)
  zT via XBAR DMA transpose        (DMA ucode, no PE/PSUM involvement)
  gates = zT.T @ W (bf16)          (TensorE, 4 matmuls of 2048 cols, PSUM x2)
  pre = gates + bias               (DVE, one [128,2048] op, drains PSUM)
  sigmoid(f,i,o) / tanh(g) -> bf16 (ACT, two ops)
  prod = si*tg                     (GpSimd)
  c = sf + prod                    (DVE)
  tanh(c)                          (ACT)
  h = tc*so                        (GpSimd)
Engine budget/tile ~ PE 3.4us > ACT 3.0 > DVE 2.9 > GpSimd 2.2 > Sync 2.5.
"""

import os
import sys

if "/opt/trn_rl_repo" not in sys.path:
    sys.path.insert(0, "/opt/trn_rl_repo")

import numpy as np

import concourse.bacc as bacc
import concourse.mybir as mybir
import concourse.tile as tile

N_CORES = 8
B, H = 65536, 512
B_CORE = B // N_CORES  # 8192
F32 = mybir.dt.float32
BF16 = mybir.dt.bfloat16
AF = mybir.ActivationFunctionType

NEFF_DUMP = "/tmp/lstm_kernel.neff"

# gate order in the packed weight/bias/psum layout: sigmoid gates first so one
# ACT op covers [0:1536], tanh gate last at [1536:2048]
#   slot 0: f (sigmoid), 1: i (sigmoid), 2: o (sigmoid), 3: g (tanh)



def _enable_ldw_opt():
    """Rewrite --enable-ldw-opt=false -> true in the walrus compile argv."""
    from concourse import bass_utils as _bu

    if getattr(_bu, "_ldw_opt_patched", False):
        return
    _orig = _bu.run_command

    def _patched(cmd, *a, **kw):
        cmd = [
            c.replace("--enable-ldw-opt=false", "--enable-ldw-opt=false")
            if isinstance(c, str)
            else c
            for c in cmd
        ]
        return _orig(cmd, *a, **kw)

    _bu.run_command = _patched
    _bu._ldw_opt_patched = True


def build_module(b_core=B_CORE, n_cores=N_CORES):
    _enable_ldw_opt()
    nc = bacc.Bacc(
        "TRN2",
        target_bir_lowering=False,
        debug=False,
        num_devices=n_cores,
    )
    x_d = nc.dram_tensor("x", [b_core, H], BF16, kind="ExternalInput").ap()
    s_d = nc.dram_tensor("s", [b_core, H], BF16, kind="ExternalInput").ap()
    wt = nc.dram_tensor("wt", [128, 4, 2048], BF16, kind="ExternalInput").ap()
    bias = nc.dram_tensor("bias", [128, 2048], F32, kind="ExternalInput").ap()
    out = nc.dram_tensor("out", [b_core, 2, H], BF16, kind="ExternalOutput").ap()

    T = b_core // 128  # 64 tiles
    NQ = T // 4  # 16 quads of 512 rows; xbar-transposed straight from DRAM
    PF = 3  # quad prefetch distance

    # Software pipeline; per-engine queues only see nearly-ready work:
    #   iter t:  Sync: [xbar x/s for quad t//4+PF], out(t-5)
    #            DVE:  [ztadd(quad t//4)], evacA(t-2), evacB(t-2), c-add(t-3)
    #            PE:   matmuls halfA(t-2) then halfB(t-2)
    #            ACT:  sig(t-2), tanhg(t-2), tanhc(t-3)
    #            Gp:   prod(t-2), h-mul(t-3)
    # PSUM: two half tiles [128,1024] (2 banks) x bufs=2 per half = 8 banks.
    # Each half's evac completes during the other half's matmuls -> PE never
    # waits on a drain even with one-slot-late WAR semantics.
    with tile.TileContext(nc) as tc:
        with (
            tc.tile_pool(name="const", bufs=1) as cpool,
            tc.tile_pool(name="pxt", bufs=5) as pxt,
            tc.tile_pool(name="pzt", bufs=3) as pzt,
            tc.tile_pool(name="pmid", bufs=4) as pmid,
            tc.tile_pool(name="pout", bufs=6) as pout,
            tc.tile_pool(name="pgates", bufs=1, space="PSUM") as pg,
        ):
            wt_sb = cpool.tile([128, 4, 2048], BF16)
            nc.sync.dma_start(out=wt_sb[:], in_=wt[:])
            bias_sb = cpool.tile([128, 2048], F32)
            nc.sync.dma_start(out=bias_sb[:], in_=bias[:])

            xt_q = [None] * NQ
            st_q = [None] * NQ
            zt_q = [None] * NQ
            acts_t = [None] * T
            prod_t = [None] * T
            ch_t = [None] * T

            def xbar_quad(q):
                # xt[p, c, b] = x[q*512 + b, c*128 + p]
                qrows = slice(q * 512, (q + 1) * 512)
                xt_q[q] = pxt.tile([128, 4, 512], BF16, tag="xt", name=f"xt{q}")
                nc.sync.dma_start_transpose(xt_q[q][:], x_d[qrows, :])
                st_q[q] = pxt.tile([128, 4, 512], BF16, tag="st", name=f"st{q}")
                nc.sync.dma_start_transpose(st_q[q][:], s_d[qrows, :])

            for q in range(min(PF, NQ)):
                xbar_quad(q)

            for t in range(T + 5):
                # S0: prefetch transposed x/s for quad t//4+PF (no upstream deps)
                if t < T and t % 4 == 0 and t // 4 + PF < NQ:
                    xbar_quad(t // 4 + PF)

                # S1: zT = xT + sT for quad t//4 (one [128,2048] bf16 2x op)
                if t < T and t % 4 == 0:
                    qa = t // 4
                    zt_q[qa] = pzt.tile([128, 4, 512], BF16, tag="zt", name=f"zt{qa}")
                    nc.vector.tensor_add(zt_q[qa][:], xt_q[qa][:], st_q[qa][:])

                # S2: matmuls + psum drain + activations for tile t-2
                if 2 <= t < T + 2:
                    m = t - 2
                    zt = zt_q[m // 4]
                    boff = (m % 4) * 128  # batch offset within the quad
                    pre = pmid.tile([128, 2048], BF16, tag="pre")
                    acts_t[m] = pmid.tile(
                        [128, 2048], BF16, tag="acts", name=f"acts{m}"
                    )
                    # alternating psum tags (bufs=1 each): explicit 2-tile WAR
                    # distance; k-outer gives runs of 4 same-stationary matmuls
                    ps = [
                        pg.tile(
                            [128, 1024], F32, tag=f"g{h}_{m % 2}", name=f"g{h}_{m}"
                        )
                        for h in range(2)
                    ]
                    for k in range(4):
                        for gs in range(4):
                            nc.tensor.matmul(
                                ps[gs // 2][:, (gs % 2) * 512 : (gs % 2 + 1) * 512],
                                zt[:, k, boff : boff + 128],
                                wt_sb[:, k, gs * 512 : (gs + 1) * 512],
                                start=(k == 0),
                                stop=(k == 3),
                            )
                    for half in range(2):
                        cols = slice(half * 1024, (half + 1) * 1024)
                        nc.vector.tensor_add(
                            pre[:, cols], ps[half][:], bias_sb[:, cols]
                        )
                    nc.scalar.activation(
                        acts_t[m][:, 0:1536], pre[:, 0:1536], AF.Sigmoid
                    )
                    nc.scalar.activation(
                        acts_t[m][:, 1536:2048], pre[:, 1536:2048], AF.Tanh
                    )
                    prod_t[m] = pmid.tile([128, H], BF16, tag="prod", name=f"prod{m}")
                    nc.gpsimd.tensor_mul(
                        prod_t[m][:], acts_t[m][:, 512:1024], acts_t[m][:, 1536:2048]
                    )

                # S3: tail for tile t-3 (prod long ready)
                if 3 <= t < T + 3:
                    e = t - 3
                    a = acts_t[e]
                    ch_t[e] = pout.tile([128, 2, H], BF16, tag="ch", name=f"ch{e}")
                    nc.vector.tensor_add(ch_t[e][:, 0, :], a[:, 0:512], prod_t[e][:])
                    tc_t = pmid.tile([128, H], BF16, tag="tc")
                    nc.scalar.activation(tc_t[:], ch_t[e][:, 0, :], AF.Tanh)
                    nc.gpsimd.tensor_mul(ch_t[e][:, 1, :], tc_t[:], a[:, 1024:1536])

                # S4: write out tile t-5
                if t >= 5:
                    o = t - 5
                    orows = slice(o * 128, (o + 1) * 128)
                    nc.sync.dma_start(out=out[orows, :, :], in_=ch_t[o][:])

    nc.compile()
    return nc


def pack_inputs(inputs, short_term_memory, Wf, bf, Wi, bi, Wg, bg, Wo, bo):
    import ml_dtypes

    bf16 = ml_dtypes.bfloat16
    x = np.asarray(inputs, np.float32).astype(bf16)
    s = np.asarray(short_term_memory, np.float32).astype(bf16)
    Ws = [Wf, Wi, Wo, Wg]
    bs = [bf, bi, bo, bg]
    wt = np.empty((128, 4, 2048), bf16)
    for gs, W in enumerate(Ws):
        Wt = np.ascontiguousarray(np.asarray(W, np.float32).T)  # [h, j] = W[j, h]
        # wt[p, kc, gs*512 + j] = W[j, kc*128 + p]
        wt[:, :, gs * 512 : (gs + 1) * 512] = (
            Wt.reshape(4, 128, 512).transpose(1, 0, 2).astype(bf16)
        )
    bias = np.empty((128, 2048), np.float32)
    for gs, b in enumerate(bs):
        bias[:, gs * 512 : (gs + 1) * 512] = np.asarray(b, np.float32)[None, :]
    return {"x": x, "s": s, "wt": wt, "bias": bias}


class Runner:
    """Compiles the module once and keeps a reusable jitted executor."""

    def __init__(self, nc=None, n_cores=N_CORES):
        import jax
        from concourse import bass2jax as b2j

        self.jax = jax
        self.n_cores = n_cores
        self.nc = nc or build_module(n_cores=n_cores)
        b2j.install_neuronx_cc_hook()

        # dump the final (renamed) NEFF so neuron-profile can pair it with NTFFs
        if not getattr(b2j, "_neff_dump_patched", False):
            orig = b2j.rename_neff_tensors_and_patch_header

            def _patched(neff_path, mapping):
                data = orig(neff_path, mapping)
                with open(NEFF_DUMP, "wb") as f:
                    f.write(data)
                return data

            b2j.rename_neff_tensors_and_patch_header = _patched
            b2j._neff_dump_patched = True

        from jax.experimental.shard_map import shard_map
        from jax.sharding import Mesh, NamedSharding, PartitionSpec

        part_name = (
            self.nc.partition_id_tensor.name if self.nc.partition_id_tensor else None
        )
        in_names, out_names, out_avals = [], [], []
        self.out_shapes = {}
        for alloc in self.nc.m.functions[0].allocations:
            if not isinstance(alloc, mybir.MemoryLocationSet):
                continue
            name = alloc.memorylocations[0].name
            if alloc.kind == "ExternalInput":
                if name != part_name:
                    in_names.append(name)
            elif alloc.kind == "ExternalOutput":
                out_names.append(name)
                shape = tuple(alloc.tensor_shape)
                dt = mybir.dt.np(alloc.dtype)
                out_avals.append(jax.core.ShapedArray(shape, dt))
                self.out_shapes[name] = (shape, dt)
        self.in_names, self.out_names = in_names, out_names
        nc_ref = self.nc

        bind_names = list(in_names) + list(out_names)
        if part_name is not None:
            bind_names.append(part_name)

        def _body(*args):
            operands = list(args)
            if part_name is not None:
                operands.append(b2j.partition_id_tensor())
            outs = b2j._bass_exec_p.bind(
                *operands,
                out_avals=tuple(out_avals),
                in_names=tuple(bind_names),
                out_names=tuple(out_names),
                lowering_input_output_aliases=(),
                sim_require_finite=False,
                sim_require_nnan=False,
                nc=nc_ref,
            )
            return tuple(outs)

        devices = jax.devices()[: self.n_cores]
        mesh = Mesh(np.asarray(devices), ("core",))
        spec = PartitionSpec("core")
        n_args = len(in_names) + len(out_names)
        self.sharding = NamedSharding(mesh, spec)
        self.fn = jax.jit(
            shard_map(
                _body,
                mesh=mesh,
                in_specs=(spec,) * n_args,
                out_specs=(spec,) * len(out_names),
                check_rep=False,
            ),
            keep_unused=True,
        )
        self._dev_args = None

    def stage(self, packed):
        """Transfer inputs (sharded/replicated as needed) to devices once."""
        jax = self.jax
        nc_n = self.n_cores
        args = []
        for name in self.in_names:
            a = packed[name]
            if name in ("x", "s"):
                glob = a  # [B, H]; axis-0 shard = per-core [B_CORE, H]
            else:
                glob = np.concatenate([a] * nc_n, axis=0)  # replicate
            args.append(glob)
        for name in self.out_names:
            shape, dt = self.out_shapes[name]
            args.append(np.zeros((shape[0] * nc_n,) + shape[1:], dt))
        self._dev_args = [jax.device_put(a, self.sharding) for a in args]

    def execute(self):
        outs = self.fn(*self._dev_args)
        self.jax.block_until_ready(outs)
        return outs

    def run(self, packed):
        self.stage(packed)
        outs = self.execute()
        res = {}
        for name, arr in zip(self.out_names, outs):
            a = np.asarray(arr)  # [n_cores*d0, ...]
            shape, _ = self.out_shapes[name]
            res[name] = a.reshape((self.n_cores, shape[0]) + tuple(shape[1:]))
        return res


_RUNNER = None


def _get_runner():
    global _RUNNER
    if _RUNNER is None:
        _RUNNER = Runner()
    return _RUNNER


def kernel(**inputs):
    r = _get_runner()
    packed = pack_inputs(**inputs)
    res = r.run(packed)
    per_core = res["out"]  # [8, 8192, 2, 512] bf16
    return np.ascontiguousarray(
        per_core.transpose(2, 0, 1, 3).reshape(2, B, H).astype(np.float32)
    )


if __name__ == "__main__":
    nc = build_module()
    print("module built + compiled OK")
